# revision 4
# baseline (speedup 1.0000x reference)
"""GCN layer (SpMM) Bass kernel v2 for 8 trn2 NeuronCores.

out[i] = sum_{e: rows[e]==i} edge_vals[e] * embeds[cols[e]]
N=100000 nodes, E=1000000 edges, D=64 features.

v2 strategy (vs baseline's per-chunk indirect_dma_start):
- rows partitioned across 8 cores (disjoint outputs, no collectives)
- per core, destination blocks of 128 rows; windows of 8 blocks (one PSUM
  bank per block)
- cols split into 4 groups of <=25000 so indices fit dma_gather's int16
- edges ordered (window, group, block, chunk); per (window, group) one
  batched dma_gather (256B/row) instead of hundreds of tiny indirect DMAs
- one-hot scatter matrices built on DVE in bf16 (batched is_equal), embs
  scaled+cast to bf16 on DVE, segment-sum via PE matmul into per-block
  PSUM accumulated across the 4 group passes
- per window: PSUM -> SBUF copies and one DMA to the output rows
"""

import sys

import numpy as np

if "/opt/trn_rl_repo" not in sys.path:
    sys.path.insert(0, "/opt/trn_rl_repo")

from ml_dtypes import bfloat16

N_NODES = 100000
D = 64
P = 128
N_CORES = 8
WBLK = 8          # blocks per window == live PSUM banks
CB = 8            # chunks per DVE batch
SEGMAX = 56       # max chunks per dma_gather call (56*128 = 7168 idxs < 8192)


def _schedule(n_nodes, rows, cols):
    """Shared (all-core) chunk schedule + per-edge slot assignment.

    Returns dict with the schedule and per-core packed arrays.
    """
    npc = n_nodes // N_CORES
    n_blocks = -(-npc // P)
    n_windows = -(-n_blocks // WBLK)
    GS = -(-n_nodes // 4)
    assert GS <= 32767

    k_of = rows // npc
    lr_of = rows - k_of * npc
    b_of = lr_of // P
    g_of = cols // GS

    cnt = np.bincount(
        (k_of * n_blocks + b_of) * 4 + g_of, minlength=N_CORES * n_blocks * 4
    ).reshape(N_CORES, n_blocks, 4)
    cnt_max = cnt.max(axis=0)  # [n_blocks, 4]
    chunks = -(-cnt_max // P)  # ceil
    chunks[:, 0] = np.maximum(chunks[:, 0], 1)  # every block initialized

    # stream order: (w, g, b)
    cell_order = []  # (b, g) in stream order
    for w in range(n_windows):
        bs = range(w * WBLK, min((w + 1) * WBLK, n_blocks))
        for g in range(4):
            for b in bs:
                cell_order.append((b, g))
    cell_pos = np.empty((n_blocks, 4), np.int64)
    cell_chunks = np.empty(len(cell_order), np.int64)
    for i, (b, g) in enumerate(cell_order):
        cell_pos[b, g] = i
        cell_chunks[i] = chunks[b, g]
    chunk_start = np.concatenate([[0], np.cumsum(cell_chunks)])  # per cell pos
    n_chunks = int(chunk_start[-1])
    S = n_chunks * P

    # per-chunk metadata: block slot + first/last flags
    chunk_block = np.empty(n_chunks, np.int64)
    for i, (b, g) in enumerate(cell_order):
        chunk_block[chunk_start[i] : chunk_start[i + 1]] = b
    first_chunk = np.full(n_blocks, -1, np.int64)
    last_chunk = np.zeros(n_blocks, np.int64)
    for c in range(n_chunks):
        b = chunk_block[c]
        if first_chunk[b] < 0:
            first_chunk[b] = c
        last_chunk[b] = c

    # gather segments: per (w, g) runs split to <= SEGMAX chunks
    segments = []  # (g, chunk0, n_chunks_in_call)
    window_chunk0 = []  # first chunk of each window
    ci = 0
    for w in range(n_windows):
        nb_w = min(WBLK, n_blocks - w * WBLK)
        window_chunk0.append(int(chunk_start[ci]))
        for g in range(4):
            seg = int(sum(cell_chunks[ci : ci + nb_w]))
            c0 = int(chunk_start[ci])
            ci += nb_w
            while seg > 0:
                take = min(seg, SEGMAX)
                segments.append((g, c0, take))
                c0 += take
                seg -= take
    window_chunk0.append(n_chunks)

    # per-core slot assignment
    idx16 = np.zeros((N_CORES, S), np.int16)
    vals = np.zeros((N_CORES, S), np.float32)
    rrow = np.full((N_CORES, S), -1.0, np.float32)
    for k in range(N_CORES):
        m = np.flatnonzero(k_of == k)
        cp = cell_pos[b_of[m], g_of[m]]
        o = np.argsort(cp, kind="stable")
        m = m[o]
        cp = cp[o]
        # rank within equal-cp runs
        starts = np.r_[0, np.flatnonzero(np.diff(cp)) + 1]
        lens = np.diff(np.r_[starts, len(cp)])
        ranks = np.arange(len(cp)) - np.repeat(starts, lens)
        slots = chunk_start[cp] * P + ranks
        idx16[k, slots] = (cols[m] - g_of[m] * GS).astype(np.int16)
        vals[k, slots] = 1.0  # overwritten by caller with real vals
        rrow[k, slots] = (lr_of[m] - b_of[m] * P).astype(np.float32)
    return dict(
        npc=npc, n_blocks=n_blocks, n_windows=n_windows, GS=GS,
        n_chunks=n_chunks, S=S, segments=segments,
        chunk_block=chunk_block, first_chunk=first_chunk, last_chunk=last_chunk,
        idx16=idx16, vals=vals, rrow=rrow, k_of=k_of,
        cell_pos=cell_pos, chunk_start=chunk_start, b_of=b_of, g_of=g_of,
        window_chunk0=window_chunk0,
    )


def _build_program(n_nodes, sched, repeats=1):
    import concourse.bacc as bacc
    from concourse import mybir
    import concourse.tile as tile

    n_blocks = sched["n_blocks"]
    n_windows = sched["n_windows"]
    GS = sched["GS"]
    n_chunks = sched["n_chunks"]
    S = sched["S"]
    segments = sched["segments"]
    chunk_block = sched["chunk_block"]
    first_chunk = sched["first_chunk"]
    last_chunk = sched["last_chunk"]

    nc = bacc.Bacc(
        "TRN2",
        target_bir_lowering=False,
        debug=False,
        num_devices=N_CORES,
        num_swdge_queues=4,
    )
    f32, bf16, i16 = mybir.dt.float32, mybir.dt.bfloat16, mybir.dt.int16
    embeds_t = nc.dram_tensor("embeds", [n_nodes, D], f32, kind="ExternalInput")
    idx_t = nc.dram_tensor("idx16", [P, S // 16], i16, kind="ExternalInput")
    vals_t = nc.dram_tensor("vals_p", [P, n_chunks], f32, kind="ExternalInput")
    rrow_t = nc.dram_tensor("rrow_p", [P, n_chunks], bf16, kind="ExternalInput")
    iota_t = nc.dram_tensor("iota", [P, CB * P], bf16, kind="ExternalInput")
    out_t = nc.dram_tensor("out", [n_blocks * P, D], f32, kind="ExternalOutput")

    with tile.TileContext(nc) as tc:
        with (
            tc.tile_pool(name="static", bufs=1) as sp,
            tc.tile_pool(name="gp", bufs=7) as gp,
            tc.tile_pool(name="ohp", bufs=4) as ohp,
            tc.tile_pool(name="ebp", bufs=4) as ebp,
            tc.tile_pool(name="outp", bufs=2) as outp,
            tc.tile_pool(name="psp", bufs=WBLK, space="PSUM") as psp,
        ):
            idx_sb = sp.tile([P, S // 16], i16)
            vals_sb = sp.tile([P, n_chunks], f32)
            rrow_sb = sp.tile([P, n_chunks], bf16)
            iota_sb = sp.tile([P, CB * P], bf16)
            nc.sync.dma_start(out=iota_sb[:], in_=iota_t[:])
            # split metadata loads at window boundaries so the first
            # gathers/matmuls don't wait for the full-stream load
            wc0 = sched["window_chunk0"]
            for w in range(n_windows):
                a, b = wc0[w], wc0[w + 1]
                nc.sync.dma_start(
                    out=idx_sb[:, a * 8 : b * 8], in_=idx_t[:, a * 8 : b * 8]
                )
                nc.sync.dma_start(out=rrow_sb[:, a:b], in_=rrow_t[:, a:b])
                nc.sync.dma_start(out=vals_sb[:, a:b], in_=vals_t[:, a:b])

            for _rep in range(repeats):
                psum_tiles = [None] * WBLK
                out_sb = None
                si = 0
                qn = 0
                for w in range(n_windows):
                    nb_w = min(WBLK, n_blocks - w * WBLK)
                    out_sb = outp.tile([P, nb_w * D], f32, name="osb")
                    # all segments of this window
                    while si < len(segments):
                        g, c0, cs = segments[si]
                        if chunk_block[c0] // WBLK != w:
                            break
                        si += 1
                        gt = gp.tile([P, cs, D], f32, name="gt")
                        nc.gpsimd.dma_gather(
                            out_ap=gt[:],
                            in_ap=embeds_t[g * GS : min((g + 1) * GS, n_nodes), :],
                            idxs_ap=idx_sb[:, c0 * 8 : (c0 + cs) * 8],
                            num_idxs=cs * P,
                            num_idxs_reg=cs * P,
                            elem_size=D,
                            single_packet=False,
                            queue_num=qn % 4,
                        )
                        qn += 1
                        for cbo in range(0, cs, CB):
                            cb = min(CB, cs - cbo)
                            c0b = c0 + cbo
                            oh = ohp.tile([P, cb * P], bf16, name="oh")
                            nc.vector.tensor_tensor(
                                out=oh[:].rearrange("p (c r) -> p c r", c=cb),
                                in0=rrow_sb[:, c0b : c0b + cb].to_broadcast(
                                    [P, cb, P]
                                ),
                                in1=iota_sb[:, : cb * P].rearrange(
                                    "p (c r) -> p c r", c=cb
                                ),
                                op=mybir.AluOpType.is_equal,
                            )
                            eb = ebp.tile([P, cb * D], bf16, name="eb")
                            nc.vector.tensor_tensor(
                                out=eb[:].rearrange("p (c f) -> p c f", c=cb),
                                in0=gt[:, cbo : cbo + cb, :],
                                in1=vals_sb[:, c0b : c0b + cb].to_broadcast(
                                    [P, cb, D]
                                ),
                                op=mybir.AluOpType.mult,
                            )
                            for c in range(cb):
                                cg = c0b + c
                                b = int(chunk_block[cg])
                                slot = b % WBLK
                                start = int(first_chunk[b]) == cg
                                stop = int(last_chunk[b]) == cg
                                if start:
                                    psum_tiles[slot] = psp.tile(
                                        [P, D], dtype=f32, space="PSUM", name="ps"
                                    )
                                nc.tensor.matmul(
                                    out=psum_tiles[slot][:],
                                    lhsT=oh[:, c * P : (c + 1) * P],
                                    rhs=eb[:, c * D : (c + 1) * D],
                                    start=start,
                                    stop=stop,
                                )
                                if stop:
                                    nc.scalar.copy(
                                        out=out_sb[:, slot * D : (slot + 1) * D],
                                        in_=psum_tiles[slot][:],
                                    )
                    # window output: SBUF [p, b, f] -> HBM rows w*WBLK*P + b*P + p
                    dst = out_t[w * WBLK * P : w * WBLK * P + nb_w * P, :]
                    dst3 = dst.rearrange("(b p) f -> p b f", b=nb_w)
                    # ACT-ring HWDGE: keeps output writes off the sync ring
                    # that streams the metadata loads
                    nc.scalar.dma_start(
                        out=dst3,
                        in_=out_sb[:].rearrange("p (b f) -> p b f", b=nb_w),
                    )
    nc.compile()
    return nc


def _kernel_impl(rows, cols, edge_vals, embeds, n_nodes, trace=False, repeats=1):
    from concourse.bass_utils import run_bass_kernel_spmd

    rows = np.asarray(rows).astype(np.int64)
    cols = np.asarray(cols).astype(np.int64)
    vs_all = np.asarray(edge_vals).astype(np.float32)
    embeds = np.ascontiguousarray(np.asarray(embeds), dtype=np.float32)

    sched = _schedule(n_nodes, rows, cols)
    npc = sched["npc"]
    S = sched["S"]
    n_chunks = sched["n_chunks"]

    # fill real edge values into the slot layout (recompute slots like _schedule)
    k_of, b_of, g_of = sched["k_of"], sched["b_of"], sched["g_of"]
    cell_pos, chunk_start = sched["cell_pos"], sched["chunk_start"]
    vals = sched["vals"]
    vals[:] = 0.0
    for k in range(N_CORES):
        m = np.flatnonzero(k_of == k)
        cp = cell_pos[b_of[m], g_of[m]]
        o = np.argsort(cp, kind="stable")
        m = m[o]
        cp = cp[o]
        starts = np.r_[0, np.flatnonzero(np.diff(cp)) + 1]
        lens = np.diff(np.r_[starts, len(cp)])
        ranks = np.arange(len(cp)) - np.repeat(starts, lens)
        slots = chunk_start[cp] * P + ranks
        vals[k, slots] = vs_all[m]

    # device layouts
    idx_dev = np.zeros((N_CORES, P, S // 16), np.int16)
    s = np.arange(S)
    for j in range(8):
        idx_dev[:, 16 * j + (s % 16), s // 16] = sched["idx16"]
    vals_dev = np.zeros((N_CORES, P, n_chunks), np.float32)
    vals_dev[:, s % P, s // P] = vals
    rrow_dev = np.zeros((N_CORES, P, n_chunks), np.float32)
    rrow_dev[:, s % P, s // P] = sched["rrow"]
    rrow_dev = rrow_dev.astype(bfloat16)
    iota = np.tile(np.arange(P, dtype=np.float32), (P, CB)).astype(bfloat16)

    nc = _build_program(n_nodes, sched, repeats=repeats)
    in_maps = [
        {
            "embeds": embeds,
            "idx16": idx_dev[k],
            "vals_p": vals_dev[k],
            "rrow_p": rrow_dev[k],
            "iota": iota,
        }
        for k in range(N_CORES)
    ]
    global _LAST
    _LAST = (nc, in_maps)
    r = run_bass_kernel_spmd(nc, in_maps, list(range(N_CORES)), trace=trace)
    out = np.concatenate(
        [r.results[k]["out"][:npc] for k in range(N_CORES)], axis=0
    ).astype(np.float32)
    if trace:
        return out, r
    return out


_LAST = None


def kernel(rows, cols, edge_vals, embeds):
    return _kernel_impl(rows, cols, edge_vals, embeds, N_NODES)
